# revision 1
# baseline (speedup 1.0000x reference)
"""Causal self-attention TRN2 kernel: build + host glue. (v4)

Sharding: tensor-parallel over heads. 16 heads / 8 cores = 2 heads per core.
Each core computes q/k/v for its 2 heads over all 4x2048 tokens, runs causal
attention, and produces a partial output projection outT [1024, 8192]
(wp rows for its heads only). Host sums the 8 partials and transposes.

All matmul operands are fp16 (1 cycle/row, FWL weight loads, fp32 PSUM
accumulation). Softmax math stays fp32 in PSUM.

v4 structure:
- Scores for BOTH heads of one key tile go into one [128, 1024] psum tile
  via two row-tiled matmuls (head A uses PE rows 0-63, head B rows 64-127,
  concurrent), so one ACT exp covers both heads.
- The AV stationary is v_aug [128, 128]: 64 v dims + 64 ones columns, so
  yt rows 64..127 hold the softmax denominator replicated 64x; the
  normalize is then a single partition-shifted DVE divide (no reciprocal,
  no broadcast).
- PE is in-order: AV matmuls are emitted one key-tile behind the scores/exp
  of the current key tile; the output projection of qtile j is deferred
  into the next qtile's instruction stream.
- Rope: one ACT copy stages the qkv psum to SBUF; the swap32 copies and
  mul/mul/add then run on DVE at SBUF speeds.
"""

from contextlib import ExitStack

import numpy as np

import concourse.bacc as bacc
import concourse.bass as bass
import concourse.mybir as mybir
import concourse.tile as tile

F32 = mybir.dt.float32
FP16 = mybir.dt.float16
AF = mybir.ActivationFunctionType
DIV = mybir.AluOpType.divide

D = 1024
H = 16
DH = 64
S = 2048
B = 4
NCORE = 8
HPC = 2  # heads per core
NT = S // 512  # 4 token tiles per batch
NKT = S // 128  # 16 key tiles per batch


def build(nb=B):
    nc = bacc.Bacc("TRN2")
    xT = nc.dram_tensor("xT", [D, B * S], FP16, kind="ExternalInput")
    wq = nc.dram_tensor("wq", [D, 128], FP16, kind="ExternalInput")
    wk = nc.dram_tensor("wk", [D, 128], FP16, kind="ExternalInput")
    wv = nc.dram_tensor("wv", [D, 128], FP16, kind="ExternalInput")
    wp = nc.dram_tensor("wp", [128, D], FP16, kind="ExternalInput")
    cs1 = nc.dram_tensor("cs1", [128, S], F32, kind="ExternalInput")
    cs2 = nc.dram_tensor("cs2", [128, S], F32, kind="ExternalInput")
    masks = nc.dram_tensor("masks", [4, 128, 512], FP16, kind="ExternalInput")
    ident = nc.dram_tensor("ident", [128, 128], FP16, kind="ExternalInput")
    ones1 = nc.dram_tensor("ones1", [128, 64], FP16, kind="ExternalInput")
    outT = nc.dram_tensor("outT", [D, B * S], F32, kind="ExternalOutput")

    with tile.TileContext(nc) as tc, ExitStack() as ctx, nc.allow_low_precision(
        reason="fp16 matmul operands with fp32 accumulation; adequate accuracy"
    ):
        const = ctx.enter_context(tc.tile_pool(name="const", bufs=1))
        xt_pool = ctx.enter_context(tc.tile_pool(name="xt", bufs=4))
        slab = ctx.enter_context(tc.tile_pool(name="slab", bufs=3))
        tmp_pool = ctx.enter_context(tc.tile_pool(name="tmp", bufs=3))
        ex_pool = ctx.enter_context(tc.tile_pool(name="ex", bufs=4))
        ytn_pool = ctx.enter_context(tc.tile_pool(name="ytn", bufs=3))
        ob_pool = ctx.enter_context(tc.tile_pool(name="ob", bufs=4))
        # PSUM budget (8 banks): sc 2x[128,1024]=4, qkv/proj/transpose 2x=2,
        # ytA/ytB 2x=2
        ps_sc = ctx.enter_context(tc.tile_pool(name="ps_sc", bufs=2, space="PSUM"))
        ps_qp = ctx.enter_context(tc.tile_pool(name="ps_qp", bufs=2, space="PSUM"))
        ps_yt = ctx.enter_context(tc.tile_pool(name="ps_yt", bufs=2, space="PSUM"))

        # ---- constants ----
        wq_sb = const.tile([128, 8, 128], FP16)
        wk_sb = const.tile([128, 8, 128], FP16)
        wv_sb = const.tile([128, 8, 128], FP16)
        for dt in range(8):
            nc.sync.dma_start(out=wq_sb[:, dt, :], in_=wq[bass.ts(dt, 128), :])
            nc.sync.dma_start(out=wk_sb[:, dt, :], in_=wk[bass.ts(dt, 128), :])
            nc.sync.dma_start(out=wv_sb[:, dt, :], in_=wv[bass.ts(dt, 128), :])
        wp_sb = const.tile([128, D], FP16)
        nc.sync.dma_start(out=wp_sb[:], in_=wp[:])
        cs1_sb = const.tile([128, S], F32)
        nc.sync.dma_start(out=cs1_sb[:], in_=cs1[:])
        cs2_sb = const.tile([128, S], F32)
        nc.sync.dma_start(out=cs2_sb[:], in_=cs2[:])
        mask_sb = const.tile([128, 4, 512], FP16)
        for d in range(4):
            nc.sync.dma_start(out=mask_sb[:, d, :], in_=masks[d])
        id_sb = const.tile([128, 128], FP16)
        nc.sync.dma_start(out=id_sb[:], in_=ident[:])
        ones_sb = const.tile([128, 64], FP16)
        nc.sync.dma_start(out=ones_sb[:], in_=ones1[:])

        def rope(dst_slice, src_ps, t, nm):
            """dst(fp16) = src*cs1_t + swap32(src)*cs2_t ; src is [128,512] psum."""
            cs1_t = cs1_sb[:, bass.ts(t, 512)]
            cs2_t = cs2_sb[:, bass.ts(t, 512)]
            qs = tmp_pool.tile([128, 512], F32, tag="qs", name="qs")
            nc.scalar.copy(qs[:], src_ps[:])
            sw = tmp_pool.tile([128, 512], F32, tag="sw", name="sw")
            for blk in range(4):
                src_blk = blk + (1 if blk % 2 == 0 else -1)
                nc.vector.tensor_copy(
                    sw[bass.ts(blk, 32), :], qs[bass.ts(src_blk, 32), :]
                )
            t1 = tmp_pool.tile([128, 512], F32, tag="t1", name="t1")
            t2 = tmp_pool.tile([128, 512], F32, tag="t2", name="t2")
            nc.vector.tensor_mul(t1[:], qs[:], cs1_t)
            nc.vector.tensor_mul(t2[:], sw[:], cs2_t)
            nc.vector.tensor_add(dst_slice, t1[:], t2[:])

        pending = []  # deferred emission closures (proj of previous qtile)

        def flush_pending():
            while pending:
                pending.pop(0)()

        for b in range(nb):
            tok0 = b * S
            qT = slab.tile([128, S], FP16, tag="qT", name="qT")
            kT = slab.tile([128, S], FP16, tag="kT", name="kT")
            # per (head, key tile): [128, 128] = [v dims (64) | ones (64)]
            v_sb = slab.tile([128, HPC * NKT * 128], FP16, tag="v_sb", name="v_sb")
            # pre-fill the ones half of every v tile: [128, tile, 64]
            v3d = v_sb[:].rearrange("p (n c) -> p n c", c=128)
            nc.vector.tensor_copy(
                v3d[:, 0 : HPC * NKT, 64:128],
                ones_sb[:, None, :].broadcast_to([128, HPC * NKT, 64]),
            )

            # ---- phase 1 + attention staircase: qtile j only needs
            # key/token tiles 0..j, so attention(j) follows phase1(t=j) ----
            def phase1(t):
                xt8 = xt_pool.tile([128, 8, 512], FP16, tag="xt", name="xt8b")
                for dt in range(8):
                    nc.sync.dma_start(
                        out=xt8[:, dt, :],
                        in_=xT[bass.ts(dt, 128), tok0 + t * 512 : tok0 + (t + 1) * 512],
                    )
                xts = [xt8[:, dt, :] for dt in range(8)]
                q_ps = ps_qp.tile([128, 512], F32, tag="qp", name="q_ps")
                for dt in range(8):
                    nc.tensor.matmul(q_ps[:], wq_sb[:, dt, :], xts[dt][:],
                                     start=dt == 0, stop=dt == 7)
                flush_pending()
                k_ps = ps_qp.tile([128, 512], F32, tag="qp", name="k_ps")
                for dt in range(8):
                    nc.tensor.matmul(k_ps[:], wk_sb[:, dt, :], xts[dt][:],
                                     start=dt == 0, stop=dt == 7)
                rope(qT[:, bass.ts(t, 512)], q_ps, t, f"q{b}_{t}")
                v_ps = ps_qp.tile([128, 512], F32, tag="qp", name="v_ps")
                for dt in range(8):
                    nc.tensor.matmul(v_ps[:], wv_sb[:, dt, :], xts[dt][:],
                                     start=dt == 0, stop=dt == 7)
                rope(kT[:, bass.ts(t, 512)], k_ps, t, f"k{b}_{t}")

                vstage = tmp_pool.tile([128, 512], FP16, tag="vst", name="vstage")
                _phase1_tail(t, vstage, v_ps)

            def _phase1_tail(t, vstage, v_ps):
                nc.scalar.copy(vstage[:], v_ps[:])
                for h in range(HPC):
                    # 4 transposes into one psum tile, one copy out
                    tp4 = ps_qp.tile([128, 256], FP16, tag="qp", name="tp4")
                    for kk in range(4):
                        nc.tensor.transpose(
                            tp4[:, bass.ts(kk, 64)],
                            vstage[bass.ts(h, 64), bass.ts(kk, 128)],
                            id_sb[bass.ts(h, 64), bass.ts(h, 64)],
                        )
                    dst = v_sb[:].rearrange("p (n c) -> p n c", c=128)[
                        :, h * NKT + t * 4 : h * NKT + t * 4 + 4, 0:64
                    ]
                    nc.vector.tensor_copy(dst, tp4[:].rearrange("p (n c) -> p n c", c=64))

            def attention(j, qT=qT, kT=kT, v_sb=v_sb, tok0=tok0):
                yTn = ytn_pool.tile([128, 512], FP16, tag="ytn", name="yTn")
                nkt = 4 * (j + 1)
                yts = {}
                for h in range(HPC):
                    yts[h] = ps_yt.tile([128, 512], F32, tag="yt", name=f"yt{h}")
                exs = {}

                def av_kt(kt, last, j=j, yts=yts, exs=exs):
                    for h in range(HPC):
                        col = (h * NKT + kt) * 128
                        nc.tensor.matmul(
                            yts[h][:],
                            v_sb[:, col : col + 128],
                            exs[kt][:, bass.ts(h, 512)],
                            start=(kt == 0),
                            stop=last,
                        )

                for kt in range(nkt):
                    sc = ps_sc.tile([128, 1024], F32, tag="sc", name="sc")
                    for h in range(HPC):
                        # head A: PE rows 0-63, head B: rows 64-127 (row-tiled,
                        # concurrent); both write their own half of sc
                        nc.tensor.matmul(
                            sc[:, bass.ts(h, 512)],
                            kT[bass.ts(h, 64), bass.ts(kt, 128)],
                            qT[bass.ts(h, 64), bass.ts(j, 512)],
                            start=True,
                            stop=True,
                        )
                    ex = ex_pool.tile([128, 1024], FP16, tag="ex", name="ex")
                    nc.scalar.activation(ex[:], sc[:], AF.Exp, scale=0.125)
                    d = kt - 4 * j
                    if d >= 0:
                        ncols = 128 * (d + 1)
                        for h in range(HPC):
                            nc.vector.tensor_mul(
                                ex[:, h * 512 : h * 512 + ncols],
                                ex[:, h * 512 : h * 512 + ncols],
                                mask_sb[:, d, 0:ncols],
                            )
                    exs[kt] = ex
                    if kt > 1:
                        av_kt(kt - 2, last=False)
                av_kt(nkt - 2, last=False)
                av_kt(nkt - 1, last=True)

                for h in range(HPC):
                    # yt rows 64..127 hold the denominator (ones columns of
                    # v_aug); only one DVE input may come from PSUM, so stage
                    # the denominator rows through SBUF on the scalar engine
                    den = tmp_pool.tile([64, 512], F32, tag="den", name="den")
                    nc.scalar.copy(den[:], yts[h][64:128, :])
                    rc64 = tmp_pool.tile([64, 512], F32, tag="rc64", name="rc64")
                    nc.vector.reciprocal_approx_fast(rc64[:], den[:])
                    nc.vector.tensor_mul(
                        yTn[bass.ts(h, 64), :], yts[h][0:64, :], rc64[:]
                    )

                def proj(j=j, yTn=yTn, tok0=tok0):
                    for dt in range(8):
                        po = ps_qp.tile([128, 512], F32, tag="qp", name="po")
                        nc.tensor.matmul(
                            po[:], wp_sb[:, bass.ts(dt, 128)], yTn[:],
                            start=True, stop=True,
                        )
                        ob = ob_pool.tile([128, 512], F32, tag="ob", name="ob")
                        nc.vector.tensor_copy(ob[:], po[:])
                        nc.sync.dma_start(
                            out=outT[
                                bass.ts(dt, 128), tok0 + j * 512 : tok0 + (j + 1) * 512
                            ],
                            in_=ob[:],
                        )

                pending.append(proj)

            for t in range(NT):
                phase1(t)
                if t > 0:
                    attention(t - 1)
            pending.append(lambda att=attention: att(NT - 1))
        flush_pending()
    nc.finalize()
    return nc


# ---------------- host side ----------------

def host_prepare(x, W_qkv, W_proj):
    xf = np.ascontiguousarray(np.asarray(x, dtype=np.float32).reshape(B * S, D))
    xT = np.ascontiguousarray(xf.T.astype(np.float16))
    Wq = np.asarray(W_qkv[:, 0:D], dtype=np.float32)
    Wk = np.asarray(W_qkv[:, D : 2 * D], dtype=np.float32)
    Wv = np.asarray(W_qkv[:, 2 * D : 3 * D], dtype=np.float32)
    Wp = np.asarray(W_proj, dtype=np.float32)
    perm = np.concatenate([np.arange(0, DH, 2), np.arange(1, DH, 2)])
    half = DH // 2
    inv_freq = 1.0 / (10000.0 ** (np.arange(half, dtype=np.float64) / half))
    freqs = np.outer(np.arange(S, dtype=np.float64), inv_freq)
    cosT = np.cos(freqs).T.astype(np.float32)
    sinT = np.sin(freqs).T.astype(np.float32)
    cs1 = np.concatenate([cosT, cosT, cosT, cosT], axis=0)
    cs2 = np.concatenate([-sinT, sinT, -sinT, sinT], axis=0)
    masks = np.zeros((4, 128, 512), dtype=np.float16)
    for d in range(4):
        ii = np.arange(128)[:, None] + 128 * d
        qq = np.arange(512)[None, :]
        masks[d] = (ii <= qq).astype(np.float16)
    ident = np.eye(128, dtype=np.float16)
    in_maps = []
    for c in range(NCORE):
        hA, hB = HPC * c, HPC * c + 1

        def cols(W, h, p=None):
            w = W[:, h * DH : (h + 1) * DH]
            return w[:, p] if p is not None else w

        in_maps.append(
            {
                "xT": xT,
                "wq": np.ascontiguousarray(
                    np.concatenate([cols(Wq, hA, perm), cols(Wq, hB, perm)], axis=1)
                ).astype(np.float16),
                "wk": np.ascontiguousarray(
                    np.concatenate([cols(Wk, hA, perm), cols(Wk, hB, perm)], axis=1)
                ).astype(np.float16),
                "wv": np.ascontiguousarray(
                    np.concatenate([cols(Wv, hA), cols(Wv, hB)], axis=1)
                ).astype(np.float16),
                "wp": np.ascontiguousarray(Wp[hA * DH : (hB + 1) * DH, :]).astype(
                    np.float16
                ),
                "cs1": cs1,
                "cs2": cs2,
                "masks": masks,
                "ident": ident,
                "ones1": np.ones((128, 64), dtype=np.float16),
            }
        )
    return in_maps




def kernel(x, W_qkv, W_proj):
    """Grading entrypoint: full inputs in, full output out.

    x [4, 2048, 1024] fp32, W_qkv [1024, 3072] fp32, W_proj [1024, 1024] fp32
    -> [4, 2048, 1024] fp32
    """
    from concourse.bass_utils import run_bass_kernel_spmd

    x = np.asarray(x)
    in_maps = host_prepare(x, np.asarray(W_qkv), np.asarray(W_proj))
    nc = build()
    res = run_bass_kernel_spmd(nc, in_maps, list(range(NCORE)))
    acc = np.zeros((D, B * S), dtype=np.float64)
    for c in range(NCORE):
        acc += res.results[c]["outT"].astype(np.float64)
    return np.ascontiguousarray(acc.T.astype(np.float32)).reshape(B, S, D)


def kernel_traced(x, W_qkv, W_proj, trace=False):
    """Dev helper: also returns the BassKernelResults (exec_time_ns etc.)."""
    from concourse.bass_utils import run_bass_kernel_spmd

    in_maps = host_prepare(np.asarray(x), np.asarray(W_qkv), np.asarray(W_proj))
    nc = build()
    res = run_bass_kernel_spmd(nc, in_maps, list(range(NCORE)), trace=trace)
    acc = np.zeros((D, B * S), dtype=np.float64)
    for c in range(NCORE):
        acc += res.results[c]["outT"].astype(np.float64)
    out = np.ascontiguousarray(acc.T.astype(np.float32)).reshape(B, S, D)
    return out, res



# revision 10
# speedup vs baseline: 1.2261x; 1.2261x over previous
"""Causal self-attention TRN2 kernel (v5): batch x head-group sharding.

Sharding: 8 cores = 4 batches x 2 head-groups (8 heads each). Each core:
- computes q/k/v for its 8 heads over its batch's 2048 tokens,
- runs causal attention (4 head-pairs, PE row-tiled score pairs),
- projects through its 512 rows of W_proj -> partial outT [1024, 2048].
Host sums core pairs and concatenates batches.

v5 structure vs v4:
- 1 batch/core -> proj contraction is 512 deep: output staging drops 4x.
- v^T computed directly on PE (stationary = x token-subtile, moving = W_v):
  no transposes, no ACT vstage.
- rope: 1 DVE copy (psum->fp16) + 2 fp16 muls + 4 partition-offset fp16
  adds (DVE 4x mode); cos/sin tables in fp16, host pre-swapped so the
  block-swap folds into the adds' operand offsets.
- diagonal narrowing: for a diagonal key tile d, scores/exp/AV only cover
  q columns >= 128d (start=True zero-region semantics make the skipped
  columns read as zero where it matters; skipped ex columns are never read).
- softmax denominator via ones-columns in the AV stationary (free on PE);
  reciprocal_approx_fast reads it straight from PSUM.
- QKV/proj matmuls are fed into the ACT-bound attention inner loop in
  ~850ns units so the PE never idles waiting for exp.
"""

from collections import deque
from contextlib import ExitStack

import numpy as np

import concourse.bacc as bacc
import concourse.bass as bass
import concourse.mybir as mybir
import concourse.tile as tile

F32 = mybir.dt.float32
FP16 = mybir.dt.float16
AF = mybir.ActivationFunctionType
ts = bass.ts

D = 1024
H = 16
DH = 64
S = 2048
B = 4
NCORE = 8
HPC = 8     # heads per core
NHP = 4     # head pairs per core
NT = S // 512   # 4 token tiles
NKT = S // 128  # 16 key tiles


def build(debug=False):
    nc = bacc.Bacc("TRN2")
    xT = nc.dram_tensor("xT", [D, S], FP16, kind="ExternalInput")
    wq = nc.dram_tensor("wq", [D, 512], FP16, kind="ExternalInput")
    wk = nc.dram_tensor("wk", [D, 512], FP16, kind="ExternalInput")
    wv = nc.dram_tensor("wv", [D, 512], FP16, kind="ExternalInput")
    wp = nc.dram_tensor("wp", [512, D], FP16, kind="ExternalInput")
    cs1 = nc.dram_tensor("cs1", [128, S], FP16, kind="ExternalInput")
    cs2 = nc.dram_tensor("cs2", [128, S], FP16, kind="ExternalInput")
    tri2 = nc.dram_tensor("tri2", [128, 256], FP16, kind="ExternalInput")
    ones1 = nc.dram_tensor("ones1", [128, 64], FP16, kind="ExternalInput")
    outT = nc.dram_tensor("outT", [D, S], FP16, kind="ExternalOutput")
    if debug:
        dbg_q = nc.dram_tensor("dbg_q", [128, NHP * S], FP16, kind="ExternalOutput")
        dbg_k = nc.dram_tensor("dbg_k", [128, NHP * S], FP16, kind="ExternalOutput")
        dbg_v = nc.dram_tensor("dbg_v", [128, HPC * NKT * 128], FP16, kind="ExternalOutput")
        dbg_y = nc.dram_tensor("dbg_y", [128, 4 * NHP * 512], FP16, kind="ExternalOutput")
        dbg_ex = nc.dram_tensor("dbg_ex", [128, 8 * 1024], FP16, kind="ExternalOutput")
        dbg_yts = nc.dram_tensor("dbg_yts", [128, 1024], F32, kind="ExternalOutput")
        dbg_rc = nc.dram_tensor("dbg_rc", [64, 1024], F32, kind="ExternalOutput")

    with tile.TileContext(nc) as tc, ExitStack() as ctx, nc.allow_low_precision(
        reason="fp16 matmul operands with fp32 accumulation; adequate accuracy"
    ):
        const = ctx.enter_context(tc.tile_pool(name="const", bufs=1))
        slab = ctx.enter_context(tc.tile_pool(name="slab", bufs=1))
        xt_pool = ctx.enter_context(tc.tile_pool(name="xt", bufs=2))
        rtmp = ctx.enter_context(tc.tile_pool(name="rtmp", bufs=3))
        ex_pool = ctx.enter_context(tc.tile_pool(name="ex", bufs=4))
        ytn_pool = ctx.enter_context(tc.tile_pool(name="ytn", bufs=2))
        ob_pool = ctx.enter_context(tc.tile_pool(name="ob", bufs=4))
        rc_pool = ctx.enter_context(tc.tile_pool(name="rc", bufs=2))
        # PSUM: qkv/proj 2x[128,512]=2 banks, sc 2x[128,1024]=4, yt 1x[128,1024]=2
        ps_qkv = ctx.enter_context(tc.tile_pool(name="ps_qkv", bufs=2, space="PSUM"))
        ps_sc = ctx.enter_context(tc.tile_pool(name="ps_sc", bufs=2, space="PSUM"))
        ps_yt = ctx.enter_context(tc.tile_pool(name="ps_yt", bufs=1, space="PSUM"))

        # ---- constants ----
        wq_sb = const.tile([128, 8, 512], FP16)
        wk_sb = const.tile([128, 8, 512], FP16)
        wv_sb = const.tile([128, 8, 512], FP16)
        for c in range(8):
            nc.sync.dma_start(out=wq_sb[:, c, :], in_=wq[ts(c, 128), :])
            nc.sync.dma_start(out=wk_sb[:, c, :], in_=wk[ts(c, 128), :])
            nc.sync.dma_start(out=wv_sb[:, c, :], in_=wv[ts(c, 128), :])
        wp_sb = const.tile([128, 4, 1024], FP16)
        for c in range(4):
            nc.sync.dma_start(out=wp_sb[:, c, :], in_=wp[ts(c, 128), :])
        cs1_sb = const.tile([128, S], FP16)
        nc.sync.dma_start(out=cs1_sb[:], in_=cs1[:])
        cs2_sb = const.tile([128, S], FP16)
        nc.sync.dma_start(out=cs2_sb[:], in_=cs2[:])
        tri_sb = const.tile([128, 256], FP16)
        nc.sync.dma_start(out=tri_sb[:], in_=tri2[:])
        ones_sb = const.tile([128, 64], FP16)
        nc.sync.dma_start(out=ones_sb[:], in_=ones1[:])

        # ---- persistent slabs ----
        qT = slab.tile([128, NHP, S], FP16)   # rows: pair dims [evens32|odds32]x2
        kT = slab.tile([128, NHP, S], FP16)
        v_sb = slab.tile([128, HPC, NKT, 128], FP16)  # [keys, head, kt, 64v|64ones]
        vflat = v_sb[:].rearrange("p h k c -> p (h k) c")
        nc.vector.tensor_copy(
            vflat[:, :, 64:128],
            ones_sb[:, None, :].broadcast_to([128, HPC * NKT, 64]),
        )

        def rope(dst, ps, t):
            """dst[128,512] fp16 = qs*cs1_t + blockswap(qs)*cs2_t (all fp16;
            the swap is single-input copies — dual-SBUF-input ops must be
            partition-aligned on TRN2)."""
            qs = rtmp.tile([128, 512], FP16, tag="qs", name="qs")
            nc.vector.tensor_copy(qs[:], ps[:])
            sw = rtmp.tile([128, 512], FP16, tag="sw", name="sw")
            for blk in range(4):
                nc.vector.tensor_copy(sw[ts(blk, 32), :], qs[ts(blk ^ 1, 32), :])
            t1 = rtmp.tile([128, 512], FP16, tag="t1", name="t1")
            t2 = rtmp.tile([128, 512], FP16, tag="t2", name="t2")
            nc.vector.tensor_mul(t1[:], qs[:], cs1_sb[:, ts(t, 512)])
            nc.vector.tensor_mul(t2[:], sw[:], cs2_sb[:, ts(t, 512)])
            nc.vector.tensor_add(dst, t1[:], t2[:])

        def make_p1_units(t):
            """Phase-1 (qkv) for token tile t, as ~850ns PE units."""
            units = []
            st = {}

            def u_dma():
                xt8 = xt_pool.tile([128, 8, 512], FP16, tag="xt", name="xt8")
                st["xt"] = xt8
                for c in range(8):
                    nc.sync.dma_start(
                        out=xt8[:, c, :], in_=xT[ts(c, 128), ts(t, 512)]
                    )

            units.append(u_dma)

            def qk_units(w_sb, dstT):
                for hp in range(NHP):
                    def u_a(hp=hp):
                        ps = ps_qkv.tile([128, 512], F32, tag="qkv", name="qk_ps")
                        st[f"ps{hp}"] = ps
                        for c in range(4):
                            nc.tensor.matmul(
                                ps[:], w_sb[:, c, ts(hp, 128)], st["xt"][:, c, :],
                                start=(c == 0), stop=False,
                            )

                    def u_b(hp=hp):
                        ps = st[f"ps{hp}"]
                        for c in range(4, 8):
                            nc.tensor.matmul(
                                ps[:], w_sb[:, c, ts(hp, 128)], st["xt"][:, c, :],
                                start=False, stop=(c == 7),
                            )

                    def u_r(hp=hp):
                        rope(dstT[:, hp, ts(t, 512)], st[f"ps{hp}"], t)

                    units.extend([u_a, u_b, u_r])

            qk_units(wq_sb, qT)
            qk_units(wk_sb, kT)

            # vT chunks: per token-sub s, out [128 tokens, 512 vdims]
            for s in range(4):
                def v_a(s=s):
                    ps = ps_qkv.tile([128, 512], F32, tag="qkv", name="vT_ps")
                    st[f"vps{s}"] = ps
                    for c in range(4):
                        nc.tensor.matmul(
                            ps[:], st["xt"][:, c, ts(s, 128)], wv_sb[:, c, :],
                            start=(c == 0), stop=False,
                        )

                def v_b(s=s):
                    ps = st[f"vps{s}"]
                    for c in range(4, 8):
                        nc.tensor.matmul(
                            ps[:], st["xt"][:, c, ts(s, 128)], wv_sb[:, c, :],
                            start=False, stop=(c == 7),
                        )

                def v_c(s=s):
                    kt = t * 4 + s
                    src = st[f"vps{s}"][:].rearrange("p (h d) -> p h d", h=HPC)
                    nc.vector.tensor_copy(v_sb[:, :, kt, 0:64], src)

                units.extend([v_a, v_b, v_c])
            return units

        def pump(feed, n):
            for _ in range(n):
                if not feed:
                    return
                feed.popleft()()

        def attention(j, yTn, feed):
            nkt = 4 * (j + 1)
            tri3 = tri_sb[:].rearrange("p (h w) -> p h w", h=2)
            for hp in range(NHP):
                yts = ps_yt.tile([128, 1024], F32, tag="yt", name="yts")
                exs = {}

                def av(kt, j=j, hp=hp, yts=yts, exs=exs):
                    d = kt - 4 * j
                    off = 128 * d if d > 0 else 0
                    for h in range(2):
                        nc.tensor.matmul(
                            yts[:, h * 512 + off : (h + 1) * 512],
                            v_sb[:, 2 * hp + h, kt, :],
                            exs[kt][:, h * 512 + off : (h + 1) * 512],
                            start=(kt == 0), stop=(kt == nkt - 1),
                            skip_group_check=True,
                        )

                for kt in range(nkt):
                    d = kt - 4 * j
                    off = 128 * d if d > 0 else 0
                    sc = ps_sc.tile([128, 1024], F32, tag="sc", name="sc")
                    for h in range(2):
                        nc.tensor.matmul(
                            sc[:, h * 512 + off : (h + 1) * 512],
                            kT[ts(h, 64), hp, ts(kt, 128)],
                            qT[ts(h, 64), hp, j * 512 + off : (j + 1) * 512],
                            start=True, stop=True,
                        )
                    ex = ex_pool.tile([128, 1024], FP16, tag="ex", name="ex")
                    if off == 0:
                        nc.scalar.activation(ex[:], sc[:], AF.Exp, scale=0.125)
                    else:
                        sc3 = sc[:].rearrange("p (h w) -> p h w", h=2)
                        ex3 = ex[:].rearrange("p (h w) -> p h w", h=2)
                        nc.scalar.activation(
                            ex3[:, :, off:], sc3[:, :, off:], AF.Exp, scale=0.125
                        )
                    if d >= 0:
                        ex3 = ex[:].rearrange("p (h w) -> p h w", h=2)
                        nc.vector.tensor_mul(
                            ex3[:, :, off : off + 128],
                            ex3[:, :, off : off + 128],
                            tri3[:],
                        )
                    exs[kt] = ex
                    if debug and j == 1 and hp == 0:
                        nc.sync.dma_start(
                            out=dbg_ex[:, kt * 1024 : (kt + 1) * 1024], in_=ex[:]
                        )
                    if kt >= 2:
                        av(kt - 2)
                    pump(feed, 1)
                av(nkt - 2)
                av(nkt - 1)

                rc = rc_pool.tile([64, 1024], F32, tag="rc", name="rc")
                if debug and j == 1 and hp == 0:
                    ys = rtmp.tile([128, 1024], F32, tag="dbgys", name="dbgys")
                    nc.vector.tensor_copy(ys[:], yts[:])
                    nc.sync.dma_start(out=dbg_yts[:], in_=ys[:])
                # reciprocal_approx_fast's BITWISE_NOT seed needs exact fp32
                # bits; PSUM reads don't preserve them — stage through SBUF.
                den = rc_pool.tile([64, 1024], F32, tag="den", name="den")
                nc.vector.tensor_copy(den[:], yts[64:128, :])
                nc.vector.reciprocal_approx_fast(rc[:], den[:])
                if debug and j == 1 and hp == 0:
                    nc.sync.dma_start(out=dbg_rc[:], in_=rc[:])
                for h in range(2):
                    nc.vector.tensor_mul(
                        yTn[ts(h, 64), hp, :],
                        yts[0:64, ts(h, 512)],
                        rc[:, ts(h, 512)],
                    )

        def proj_units(j, yTn):
            units = []
            for dt in range(8):
                def u(dt=dt, yTn=yTn):
                    po = ps_qkv.tile([128, 512], F32, tag="qkv", name="po")
                    for hp in range(NHP):
                        nc.tensor.matmul(
                            po[:], wp_sb[:, hp, ts(dt, 128)], yTn[:, hp, :],
                            start=(hp == 0), stop=(hp == NHP - 1),
                        )
                    ob = ob_pool.tile([128, 512], FP16, tag="ob", name="ob")
                    nc.vector.tensor_copy(ob[:], po[:])
                    nc.sync.dma_start(
                        out=outT[ts(dt, 128), ts(j, 512)], in_=ob[:]
                    )

                units.append(u)
            return units

        # ---- main schedule ----
        yTns = {}
        for u in make_p1_units(0):
            u()
        for t in range(1, NT + 1):
            j = t - 1
            yTns[j] = ytn_pool.tile([128, NHP, 512], FP16, tag="yTn", name="yTn")
            feed = deque()
            if t <= NT - 1:
                feed.extend(make_p1_units(t))
            if j >= 1:
                feed.extend(proj_units(j - 1, yTns[j - 1]))
            attention(j, yTns[j], feed)
            pump(feed, len(feed))
        for u in proj_units(NT - 1, yTns[NT - 1]):
            u()
        if debug:
            nc.sync.dma_start(
                out=dbg_q[:], in_=qT[:].rearrange("p h s -> p (h s)")
            )
            nc.sync.dma_start(
                out=dbg_k[:], in_=kT[:].rearrange("p h s -> p (h s)")
            )
            nc.sync.dma_start(
                out=dbg_v[:], in_=v_sb[:].rearrange("p h k c -> p (h k c)")
            )
            for j in range(4):
                nc.sync.dma_start(
                    out=dbg_y[:, j * NHP * 512 : (j + 1) * NHP * 512],
                    in_=yTns[j][:].rearrange("p h s -> p (h s)"),
                )
    nc.finalize()
    return nc


# ---------------- host side ----------------

def host_prepare(x, W_qkv, W_proj):
    x = np.asarray(x, dtype=np.float32)
    Wq = np.asarray(W_qkv[:, 0:D], dtype=np.float32)
    Wk = np.asarray(W_qkv[:, D : 2 * D], dtype=np.float32)
    Wv = np.asarray(W_qkv[:, 2 * D : 3 * D], dtype=np.float32)
    Wp = np.asarray(W_proj, dtype=np.float32)
    perm = np.concatenate([np.arange(0, DH, 2), np.arange(1, DH, 2)])
    half = DH // 2
    inv_freq = 1.0 / (10000.0 ** (np.arange(half, dtype=np.float64) / half))
    freqs = np.outer(np.arange(S, dtype=np.float64), inv_freq)
    cosT = np.cos(freqs).T
    sinT = np.sin(freqs).T
    # rope: dst = qs*cs1 + swap(qs)*cs2
    cs1 = np.concatenate([cosT, cosT, cosT, cosT], axis=0).astype(np.float16)
    cs2s = np.concatenate([-sinT, sinT, -sinT, sinT], axis=0).astype(np.float16)
    ii = np.arange(128)[:, None]
    qq = np.arange(128)[None, :]
    tri = (ii <= qq).astype(np.float16)
    tri2 = np.concatenate([tri, tri], axis=1)

    def headcols(W, h, p=None):
        w = W[:, h * DH : (h + 1) * DH]
        return w[:, p] if p is not None else w

    in_maps = []
    for c in range(NCORE):
        b, hg = c // 2, c % 2
        heads = [8 * hg + i for i in range(HPC)]
        wq_c = np.concatenate([headcols(Wq, h, perm) for h in heads], axis=1)
        wk_c = np.concatenate([headcols(Wk, h, perm) for h in heads], axis=1)
        wv_c = np.concatenate([headcols(Wv, h) for h in heads], axis=1)
        wp_c = Wp[heads[0] * DH : (heads[-1] + 1) * DH, :]
        in_maps.append(
            {
                "xT": np.ascontiguousarray(x[b].T).astype(np.float16),
                "wq": np.ascontiguousarray(wq_c).astype(np.float16),
                "wk": np.ascontiguousarray(wk_c).astype(np.float16),
                "wv": np.ascontiguousarray(wv_c).astype(np.float16),
                "wp": np.ascontiguousarray(wp_c).astype(np.float16),
                "cs1": cs1,
                "cs2": cs2s,
                "tri2": tri2,
                "ones1": np.ones((128, 64), dtype=np.float16),
            }
        )
    return in_maps


def _gather(res):
    out = np.empty((B, S, D), dtype=np.float32)
    for b in range(B):
        acc = res.results[2 * b]["outT"].astype(np.float32) + res.results[
            2 * b + 1
        ]["outT"].astype(np.float32)
        out[b] = acc.T
    return out


def kernel(x, W_qkv, W_proj):
    """Grading entrypoint: full inputs in, full output out.

    x [4, 2048, 1024] fp32, W_qkv [1024, 3072] fp32, W_proj [1024, 1024] fp32
    -> [4, 2048, 1024] fp32
    """
    from concourse.bass_utils import run_bass_kernel_spmd

    in_maps = host_prepare(np.asarray(x), np.asarray(W_qkv), np.asarray(W_proj))
    nc = build()
    res = run_bass_kernel_spmd(nc, in_maps, list(range(NCORE)))
    return _gather(res)


def kernel_traced(x, W_qkv, W_proj, trace=False):
    """Dev helper: also returns the BassKernelResults (exec_time_ns etc.)."""
    from concourse.bass_utils import run_bass_kernel_spmd

    in_maps = host_prepare(np.asarray(x), np.asarray(W_qkv), np.asarray(W_proj))
    nc = build()
    res = run_bass_kernel_spmd(nc, in_maps, list(range(NCORE)), trace=trace)
    return _gather(res), res


# revision 19
# speedup vs baseline: 1.2768x; 1.0413x over previous
"""Causal self-attention TRN2 kernel (v5): batch x head-group sharding.

Sharding: 8 cores = 4 batches x 2 head-groups (8 heads each). Each core:
- computes q/k/v for its 8 heads over its batch's 2048 tokens,
- runs causal attention (4 head-pairs, PE row-tiled score pairs),
- projects through its 512 rows of W_proj -> partial outT [1024, 2048].
Host sums core pairs and concatenates batches.

v5 structure vs v4:
- 1 batch/core -> proj contraction is 512 deep: output staging drops 4x.
- v^T computed directly on PE (stationary = x token-subtile, moving = W_v):
  no transposes, no ACT vstage.
- rope: 1 DVE copy (psum->fp16) + 2 fp16 muls + 4 partition-offset fp16
  adds (DVE 4x mode); cos/sin tables in fp16, host pre-swapped so the
  block-swap folds into the adds' operand offsets.
- diagonal narrowing: for a diagonal key tile d, scores/exp/AV only cover
  q columns >= 128d (start=True zero-region semantics make the skipped
  columns read as zero where it matters; skipped ex columns are never read).
- softmax denominator via ones-columns in the AV stationary (free on PE);
  reciprocal_approx_fast reads it straight from PSUM.
- QKV/proj matmuls are fed into the ACT-bound attention inner loop in
  ~850ns units so the PE never idles waiting for exp.
"""

from collections import deque
from contextlib import ExitStack

import numpy as np

import concourse.bacc as bacc
import concourse.bass as bass
import concourse.mybir as mybir
import concourse.tile as tile

F32 = mybir.dt.float32
FP16 = mybir.dt.float16
AF = mybir.ActivationFunctionType
ts = bass.ts

D = 1024
H = 16
DH = 64
S = 2048
B = 4
NCORE = 8
HPC = 8     # heads per core
NHP = 4     # head pairs per core
NT = S // 512   # 4 token tiles
NKT = S // 128  # 16 key tiles


def build(debug=False):
    nc = bacc.Bacc("TRN2")
    xT = nc.dram_tensor("xT", [D, S], FP16, kind="ExternalInput")
    wq = nc.dram_tensor("wq", [D, 512], FP16, kind="ExternalInput")
    wk = nc.dram_tensor("wk", [D, 512], FP16, kind="ExternalInput")
    wv = nc.dram_tensor("wv", [D, 512], FP16, kind="ExternalInput")
    wp = nc.dram_tensor("wp", [512, D], FP16, kind="ExternalInput")
    cs1 = nc.dram_tensor("cs1", [128, S], FP16, kind="ExternalInput")
    cs2 = nc.dram_tensor("cs2", [128, S], FP16, kind="ExternalInput")
    tri2 = nc.dram_tensor("tri2", [128, 256], FP16, kind="ExternalInput")
    ones1 = nc.dram_tensor("ones1", [128, 64], FP16, kind="ExternalInput")
    outT = nc.dram_tensor("outT", [D, S], FP16, kind="ExternalOutput")
    if debug:
        dbg_q = nc.dram_tensor("dbg_q", [128, NHP * S], FP16, kind="ExternalOutput")
        dbg_k = nc.dram_tensor("dbg_k", [128, NHP * S], FP16, kind="ExternalOutput")
        dbg_v = nc.dram_tensor("dbg_v", [128, HPC * NKT * 128], FP16, kind="ExternalOutput")
        dbg_y = nc.dram_tensor("dbg_y", [128, 4 * NHP * 512], FP16, kind="ExternalOutput")
        dbg_ex = nc.dram_tensor("dbg_ex", [128, 8 * 1024], FP16, kind="ExternalOutput")
        dbg_yts = nc.dram_tensor("dbg_yts", [128, 1024], F32, kind="ExternalOutput")
        dbg_rc = nc.dram_tensor("dbg_rc", [64, 1024], F32, kind="ExternalOutput")

    with tile.TileContext(nc) as tc, ExitStack() as ctx, nc.allow_low_precision(
        reason="fp16 matmul operands with fp32 accumulation; adequate accuracy"
    ):
        const = ctx.enter_context(tc.tile_pool(name="const", bufs=1))
        slab = ctx.enter_context(tc.tile_pool(name="slab", bufs=1))
        xt_pool = ctx.enter_context(tc.tile_pool(name="xt", bufs=2))
        rtmp = ctx.enter_context(tc.tile_pool(name="rtmp", bufs=3))
        ex_pool = ctx.enter_context(tc.tile_pool(name="ex", bufs=4))
        ytn_pool = ctx.enter_context(tc.tile_pool(name="ytn", bufs=2))
        ob_pool = ctx.enter_context(tc.tile_pool(name="ob", bufs=4))
        rc_pool = ctx.enter_context(tc.tile_pool(name="rc", bufs=2))
        # PSUM: qkv/proj 2x[128,512]=2 banks, sc 2x[128,1024]=4, yt 1x[128,1024]=2
        ps_qkv = ctx.enter_context(tc.tile_pool(name="ps_qkv", bufs=2, space="PSUM"))
        ps_sc = ctx.enter_context(tc.tile_pool(name="ps_sc", bufs=2, space="PSUM"))
        ps_yt = ctx.enter_context(tc.tile_pool(name="ps_yt", bufs=1, space="PSUM"))

        # ---- constants ----
        wq_sb = const.tile([128, 8, 512], FP16)
        wk_sb = const.tile([128, 8, 512], FP16)
        wv_sb = const.tile([128, 8, 512], FP16)
        for c in range(8):
            nc.sync.dma_start(out=wq_sb[:, c, :], in_=wq[ts(c, 128), :])
            nc.sync.dma_start(out=wk_sb[:, c, :], in_=wk[ts(c, 128), :])
            nc.sync.dma_start(out=wv_sb[:, c, :], in_=wv[ts(c, 128), :])
        wp_sb = const.tile([128, 4, 1024], FP16)
        for c in range(4):
            nc.sync.dma_start(out=wp_sb[:, c, :], in_=wp[ts(c, 128), :])
        cs1_sb = const.tile([128, S], FP16)
        nc.sync.dma_start(out=cs1_sb[:], in_=cs1[:])
        cs2_sb = const.tile([128, S], FP16)
        nc.sync.dma_start(out=cs2_sb[:], in_=cs2[:])
        tri_sb = const.tile([128, 256], FP16)
        nc.sync.dma_start(out=tri_sb[:], in_=tri2[:])
        ones_sb = const.tile([128, 64], FP16)
        nc.sync.dma_start(out=ones_sb[:], in_=ones1[:])

        # ---- persistent slabs ----
        qT = slab.tile([128, NHP, S], FP16)   # rows: pair dims [evens32|odds32]x2
        kT = slab.tile([128, NHP, S], FP16)
        v_sb = slab.tile([128, HPC, NKT, 128], FP16)  # [keys, head, kt, 64v|64ones]
        vflat = v_sb[:].rearrange("p h k c -> p (h k) c")
        nc.vector.tensor_copy(
            vflat[:, :, 64:128],
            ones_sb[:, None, :].broadcast_to([128, HPC * NKT, 64]),
        )

        def rope(dst, ps, t):
            """dst[128,512] fp16 = qs*cs1_t + blockswap(qs)*cs2_t. The swap is
            single-input copies (dual-SBUF-input ops must be partition-aligned
            on TRN2); the second mul + final add run on the idle GpSimd."""
            qs = rtmp.tile([128, 512], FP16, tag="qs", name="qs")
            nc.vector.tensor_copy(qs[:], ps[:])
            sw = rtmp.tile([128, 512], FP16, tag="sw", name="sw")
            for blk in range(4):
                nc.vector.tensor_copy(sw[ts(blk, 32), :], qs[ts(blk ^ 1, 32), :])
            t1 = rtmp.tile([128, 512], FP16, tag="t1", name="t1")
            t2 = rtmp.tile([128, 512], FP16, tag="t2", name="t2")
            nc.vector.tensor_mul(t1[:], qs[:], cs1_sb[:, ts(t, 512)])
            nc.vector.tensor_mul(t2[:], sw[:], cs2_sb[:, ts(t, 512)])
            nc.vector.tensor_add(dst, t1[:], t2[:])

        def make_p1_units(t):
            """Phase-1 (qkv) for token tile t, split into ~850ns PE units.
            Returns (front, rest): front = dma + q/k for head-pair 0 + all
            vT subtiles (what attention(t) hp0 needs); rest = q/k for head
            pairs 1-3, pumped during attention(t)'s earlier head pairs.
            Units are (fn, has_pe_work) pairs."""
            front, rest = [], []
            st = {}

            def u_dma():
                xt8 = xt_pool.tile([128, 8, 512], FP16, tag="xt", name="xt8")
                st["xt"] = xt8
                for c in range(8):
                    nc.sync.dma_start(
                        out=xt8[:, c, :], in_=xT[ts(c, 128), ts(t, 512)]
                    )

            front.append((u_dma, False))

            def qk_units(w_sb, dstT, key, hp, out):
                def u_a(hp=hp):
                    ps = ps_qkv.tile([128, 512], F32, tag="qkv", name="qk_ps")
                    st[f"{key}{hp}"] = ps
                    for c in range(4):
                        nc.tensor.matmul(
                            ps[:], w_sb[:, c, ts(hp, 128)], st["xt"][:, c, :],
                            start=(c == 0), stop=False,
                        )

                def u_b(hp=hp):
                    ps = st[f"{key}{hp}"]
                    for c in range(4, 8):
                        nc.tensor.matmul(
                            ps[:], w_sb[:, c, ts(hp, 128)], st["xt"][:, c, :],
                            start=False, stop=(c == 7),
                        )

                def u_r(hp=hp):
                    rope(dstT[:, hp, ts(t, 512)], st[f"{key}{hp}"], t)

                out.extend([(u_a, True), (u_b, True), (u_r, False)])

            qk_units(wq_sb, qT, "q", 0, front)
            qk_units(wk_sb, kT, "k", 0, front)

            # vT chunks: per token-sub s, out [128 tokens, 512 vdims]
            for s in range(4):
                def v_a(s=s):
                    ps = ps_qkv.tile([128, 512], F32, tag="qkv", name="vT_ps")
                    st[f"vps{s}"] = ps
                    for c in range(4):
                        nc.tensor.matmul(
                            ps[:], st["xt"][:, c, ts(s, 128)], wv_sb[:, c, :],
                            start=(c == 0), stop=False,
                        )

                def v_b(s=s):
                    ps = st[f"vps{s}"]
                    for c in range(4, 8):
                        nc.tensor.matmul(
                            ps[:], st["xt"][:, c, ts(s, 128)], wv_sb[:, c, :],
                            start=False, stop=(c == 7),
                        )

                def v_c(s=s):
                    kt = t * 4 + s
                    src = st[f"vps{s}"][:].rearrange("p (h d) -> p h d", h=HPC)
                    nc.vector.tensor_copy(v_sb[:, :, kt, 0:64], src)

                front.extend([(v_a, True), (v_b, True), (v_c, False)])

            for hp in range(1, NHP):
                qk_units(wq_sb, qT, "q", hp, rest)
                qk_units(wk_sb, kT, "k", hp, rest)
            return front, rest

        consumed = [0]

        def pump(feed, n_pe):
            """Run feed units until n_pe PE-bearing units have been emitted
            (light DVE-only units don't count toward the budget)."""
            while n_pe > 0 and feed:
                fn, has_pe = feed.popleft()
                fn()
                consumed[0] += 1
                if has_pe:
                    n_pe -= 1

        def attention(j, yTn, feed, gates=None):
            nkt = 4 * (j + 1)
            tri3 = tri_sb[:].rearrange("p (h w) -> p h w", h=2)
            for hp in range(NHP):
                if gates is not None and hp >= 1:
                    # hp's scores read qT/kT[hp] of tile j, produced by carry
                    # units still in the feed — force-run them before emitting
                    # any reader (tile deps are ordered by emission).
                    while consumed[0] < gates[hp] and feed:
                        fn, _ = feed.popleft()
                        fn()
                        consumed[0] += 1
                yts = ps_yt.tile([128, 1024], F32, tag="yt", name="yts")
                exs = {}

                def av(kt, j=j, hp=hp, yts=yts, exs=exs):
                    d = kt - 4 * j
                    off = 128 * d if d > 0 else 0
                    for h in range(2):
                        nc.tensor.matmul(
                            yts[:, h * 512 + off : (h + 1) * 512],
                            v_sb[:, 2 * hp + h, kt, :],
                            exs[kt][:, h * 512 + off : (h + 1) * 512],
                            start=(kt == 0), stop=(kt == nkt - 1),
                            skip_group_check=True,
                        )

                for kt in range(nkt):
                    d = kt - 4 * j
                    off = 128 * d if d > 0 else 0
                    sc = ps_sc.tile([128, 1024], F32, tag="sc", name="sc")
                    for h in range(2):
                        nc.tensor.matmul(
                            sc[:, h * 512 + off : (h + 1) * 512],
                            kT[ts(h, 64), hp, ts(kt, 128)],
                            qT[ts(h, 64), hp, j * 512 + off : (j + 1) * 512],
                            start=True, stop=True,
                        )
                    ex = ex_pool.tile([128, 1024], FP16, tag="ex", name="ex")
                    if off == 0:
                        nc.scalar.activation(ex[:], sc[:], AF.Exp, scale=0.125)
                    else:
                        sc3 = sc[:].rearrange("p (h w) -> p h w", h=2)
                        ex3 = ex[:].rearrange("p (h w) -> p h w", h=2)
                        nc.scalar.activation(
                            ex3[:, :, off:], sc3[:, :, off:], AF.Exp, scale=0.125
                        )
                    if d >= 0:
                        ex3 = ex[:].rearrange("p (h w) -> p h w", h=2)
                        nc.vector.tensor_mul(
                            ex3[:, :, off : off + 128],
                            ex3[:, :, off : off + 128],
                            tri3[:],
                        )
                    exs[kt] = ex
                    if debug and j == 1 and hp == 0:
                        nc.sync.dma_start(
                            out=dbg_ex[:, kt * 1024 : (kt + 1) * 1024], in_=ex[:]
                        )
                    if kt >= 2:
                        av(kt - 2)
                    pump(feed, 1)
                av(nkt - 2)
                av(nkt - 1)

                rc = rc_pool.tile([64, 1024], F32, tag="rc", name="rc")
                if debug and j == 1 and hp == 0:
                    ys = rtmp.tile([128, 1024], F32, tag="dbgys", name="dbgys")
                    nc.vector.tensor_copy(ys[:], yts[:])
                    nc.sync.dma_start(out=dbg_yts[:], in_=ys[:])
                # reciprocal_approx_fast's BITWISE_NOT seed needs exact fp32
                # bits; PSUM reads don't preserve them — stage through SBUF
                # (on ACT: it has slack, DVE is loaded).
                den = rc_pool.tile([64, 1024], F32, tag="den", name="den")
                nc.scalar.copy(den[:], yts[64:128, :])
                nc.vector.reciprocal_approx_fast(rc[:], den[:])
                if debug and j == 1 and hp == 0:
                    nc.sync.dma_start(out=dbg_rc[:], in_=rc[:])
                for h in range(2):
                    nc.vector.tensor_mul(
                        yTn[ts(h, 64), hp, :],
                        yts[0:64, ts(h, 512)],
                        rc[:, ts(h, 512)],
                    )

        def proj_units(j, yTn):
            units = []
            for dt in range(8):
                def u(dt=dt, yTn=yTn):
                    po = ps_qkv.tile([128, 512], F32, tag="qkv", name="po")
                    for hp in range(NHP):
                        nc.tensor.matmul(
                            po[:], wp_sb[:, hp, ts(dt, 128)], yTn[:, hp, :],
                            start=(hp == 0), stop=(hp == NHP - 1),
                        )
                    ob = ob_pool.tile([128, 512], FP16, tag="ob", name="ob")
                    nc.scalar.copy(ob[:], po[:])
                    nc.sync.dma_start(
                        out=outT[ts(dt, 128), ts(j, 512)], in_=ob[:]
                    )

                units.append((u, True))
            return units

        # ---- main schedule ----
        yTns = {}
        front0, rest0 = make_p1_units(0)
        for fn, _ in front0:
            fn()
        carry = list(rest0)
        for t in range(1, NT + 1):
            j = t - 1
            yTns[j] = ytn_pool.tile([128, NHP, 512], FP16, tag="yTn", name="yTn")
            feed = deque(carry)
            carry = []
            if t <= NT - 1:
                front, rest = make_p1_units(t)
                feed.extend(front)
                carry = list(rest)
            if j >= 1:
                feed.extend(proj_units(j - 1, yTns[j - 1]))
            consumed[0] = 0
            # carry units sit at the front of the feed: 6 per head pair
            attention(j, yTns[j], feed, gates=[0, 6, 12, 18])
            while feed:
                fn, _ = feed.popleft()
                fn()
        for fn, _ in proj_units(NT - 1, yTns[NT - 1]):
            fn()
        if debug:
            nc.sync.dma_start(
                out=dbg_q[:], in_=qT[:].rearrange("p h s -> p (h s)")
            )
            nc.sync.dma_start(
                out=dbg_k[:], in_=kT[:].rearrange("p h s -> p (h s)")
            )
            nc.sync.dma_start(
                out=dbg_v[:], in_=v_sb[:].rearrange("p h k c -> p (h k c)")
            )
            for j in range(4):
                nc.sync.dma_start(
                    out=dbg_y[:, j * NHP * 512 : (j + 1) * NHP * 512],
                    in_=yTns[j][:].rearrange("p h s -> p (h s)"),
                )
    nc.finalize()
    return nc


# ---------------- host side ----------------

def host_prepare(x, W_qkv, W_proj):
    x = np.asarray(x, dtype=np.float32)
    Wq = np.asarray(W_qkv[:, 0:D], dtype=np.float32)
    Wk = np.asarray(W_qkv[:, D : 2 * D], dtype=np.float32)
    Wv = np.asarray(W_qkv[:, 2 * D : 3 * D], dtype=np.float32)
    Wp = np.asarray(W_proj, dtype=np.float32)
    perm = np.concatenate([np.arange(0, DH, 2), np.arange(1, DH, 2)])
    half = DH // 2
    inv_freq = 1.0 / (10000.0 ** (np.arange(half, dtype=np.float64) / half))
    freqs = np.outer(np.arange(S, dtype=np.float64), inv_freq)
    cosT = np.cos(freqs).T
    sinT = np.sin(freqs).T
    # rope: dst = qs*cs1 + swap(qs)*cs2
    cs1 = np.concatenate([cosT, cosT, cosT, cosT], axis=0).astype(np.float16)
    cs2s = np.concatenate([-sinT, sinT, -sinT, sinT], axis=0).astype(np.float16)
    ii = np.arange(128)[:, None]
    qq = np.arange(128)[None, :]
    tri = (ii <= qq).astype(np.float16)
    tri2 = np.concatenate([tri, tri], axis=1)

    def headcols(W, h, p=None):
        w = W[:, h * DH : (h + 1) * DH]
        return w[:, p] if p is not None else w

    in_maps = []
    for c in range(NCORE):
        b, hg = c // 2, c % 2
        heads = [8 * hg + i for i in range(HPC)]
        wq_c = np.concatenate([headcols(Wq, h, perm) for h in heads], axis=1)
        wk_c = np.concatenate([headcols(Wk, h, perm) for h in heads], axis=1)
        wv_c = np.concatenate([headcols(Wv, h) for h in heads], axis=1)
        wp_c = Wp[heads[0] * DH : (heads[-1] + 1) * DH, :]
        in_maps.append(
            {
                "xT": np.ascontiguousarray(x[b].T).astype(np.float16),
                "wq": np.ascontiguousarray(wq_c).astype(np.float16),
                "wk": np.ascontiguousarray(wk_c).astype(np.float16),
                "wv": np.ascontiguousarray(wv_c).astype(np.float16),
                "wp": np.ascontiguousarray(wp_c).astype(np.float16),
                "cs1": cs1,
                "cs2": cs2s,
                "tri2": tri2,
                "ones1": np.ones((128, 64), dtype=np.float16),
            }
        )
    return in_maps


def _gather(res):
    out = np.empty((B, S, D), dtype=np.float32)
    for b in range(B):
        acc = res.results[2 * b]["outT"].astype(np.float32) + res.results[
            2 * b + 1
        ]["outT"].astype(np.float32)
        out[b] = acc.T
    return out


def kernel(x, W_qkv, W_proj):
    """Grading entrypoint: full inputs in, full output out.

    x [4, 2048, 1024] fp32, W_qkv [1024, 3072] fp32, W_proj [1024, 1024] fp32
    -> [4, 2048, 1024] fp32
    """
    from concourse.bass_utils import run_bass_kernel_spmd

    in_maps = host_prepare(np.asarray(x), np.asarray(W_qkv), np.asarray(W_proj))
    nc = build()
    res = run_bass_kernel_spmd(nc, in_maps, list(range(NCORE)))
    return _gather(res)


def kernel_traced(x, W_qkv, W_proj, trace=False):
    """Dev helper: also returns the BassKernelResults (exec_time_ns etc.)."""
    from concourse.bass_utils import run_bass_kernel_spmd

    in_maps = host_prepare(np.asarray(x), np.asarray(W_qkv), np.asarray(W_proj))
    nc = build()
    res = run_bass_kernel_spmd(nc, in_maps, list(range(NCORE)), trace=trace)
    return _gather(res), res


# revision 22
# speedup vs baseline: 1.3564x; 1.0623x over previous
"""Causal self-attention TRN2 kernel (v5): batch x head-group sharding.

Sharding: 8 cores = 4 batches x 2 head-groups (8 heads each). Each core:
- computes q/k/v for its 8 heads over its batch's 2048 tokens,
- runs causal attention (4 head-pairs, PE row-tiled score pairs),
- projects through its 512 rows of W_proj -> partial outT [1024, 2048].
Host sums core pairs and concatenates batches.

v5 structure vs v4:
- 1 batch/core -> proj contraction is 512 deep: output staging drops 4x.
- v^T computed directly on PE (stationary = x token-subtile, moving = W_v):
  no transposes, no ACT vstage.
- rope: 1 DVE copy (psum->fp16) + 2 fp16 muls + 4 partition-offset fp16
  adds (DVE 4x mode); cos/sin tables in fp16, host pre-swapped so the
  block-swap folds into the adds' operand offsets.
- diagonal narrowing: for a diagonal key tile d, scores/exp/AV only cover
  q columns >= 128d (start=True zero-region semantics make the skipped
  columns read as zero where it matters; skipped ex columns are never read).
- softmax denominator via ones-columns in the AV stationary (free on PE);
  reciprocal_approx_fast reads it straight from PSUM.
- QKV/proj matmuls are fed into the ACT-bound attention inner loop in
  ~850ns units so the PE never idles waiting for exp.
"""

from collections import deque
from contextlib import ExitStack

import numpy as np

import concourse.bacc as bacc
import concourse.bass as bass
import concourse.mybir as mybir
import concourse.tile as tile

F32 = mybir.dt.float32
FP16 = mybir.dt.float16
AF = mybir.ActivationFunctionType
ts = bass.ts

D = 1024
H = 16
DH = 64
S = 2048
B = 4
NCORE = 8
HPC = 8     # heads per core
NHP = 4     # head pairs per core
NT = S // 512   # 4 token tiles
NKT = S // 128  # 16 key tiles


def build(debug=False):
    nc = bacc.Bacc("TRN2")
    xT = nc.dram_tensor("xT", [D, S], FP16, kind="ExternalInput")
    wq = nc.dram_tensor("wq", [D, 512], FP16, kind="ExternalInput")
    wk = nc.dram_tensor("wk", [D, 512], FP16, kind="ExternalInput")
    wv = nc.dram_tensor("wv", [D, 512], FP16, kind="ExternalInput")
    wp = nc.dram_tensor("wp", [512, D], FP16, kind="ExternalInput")
    cs1 = nc.dram_tensor("cs1", [128, S], FP16, kind="ExternalInput")
    cs2 = nc.dram_tensor("cs2", [128, S], FP16, kind="ExternalInput")
    tri2 = nc.dram_tensor("tri2", [128, 256], FP16, kind="ExternalInput")
    ones1 = nc.dram_tensor("ones1", [128, 64], FP16, kind="ExternalInput")
    outT = nc.dram_tensor("outT", [D, S], FP16, kind="ExternalOutput")
    if debug:
        dbg_q = nc.dram_tensor("dbg_q", [128, NHP * S], FP16, kind="ExternalOutput")
        dbg_k = nc.dram_tensor("dbg_k", [128, NHP * S], FP16, kind="ExternalOutput")
        dbg_v = nc.dram_tensor("dbg_v", [128, HPC * NKT * 128], FP16, kind="ExternalOutput")
        dbg_y = nc.dram_tensor("dbg_y", [128, 4 * NHP * 512], FP16, kind="ExternalOutput")
        dbg_ex = nc.dram_tensor("dbg_ex", [128, 8 * 1024], FP16, kind="ExternalOutput")
        dbg_yts = nc.dram_tensor("dbg_yts", [128, 1024], F32, kind="ExternalOutput")
        dbg_rc = nc.dram_tensor("dbg_rc", [64, 1024], F32, kind="ExternalOutput")

    with tile.TileContext(nc) as tc, ExitStack() as ctx, nc.allow_low_precision(
        reason="fp16 matmul operands with fp32 accumulation; adequate accuracy"
    ):
        const = ctx.enter_context(tc.tile_pool(name="const", bufs=1))
        slab = ctx.enter_context(tc.tile_pool(name="slab", bufs=1))
        xt_pool = ctx.enter_context(tc.tile_pool(name="xt", bufs=2))
        rtmp = ctx.enter_context(tc.tile_pool(name="rtmp", bufs=3))
        ex_pool = ctx.enter_context(tc.tile_pool(name="ex", bufs=4))
        ytn_pool = ctx.enter_context(tc.tile_pool(name="ytn", bufs=2))
        ob_pool = ctx.enter_context(tc.tile_pool(name="ob", bufs=4))
        rc_pool = ctx.enter_context(tc.tile_pool(name="rc", bufs=2))
        # PSUM: qkv/proj 2x[128,512]=2 banks, sc 2x[128,1024]=4, yt 1x[128,1024]=2
        ps_qkv = ctx.enter_context(tc.tile_pool(name="ps_qkv", bufs=2, space="PSUM"))
        ps_sc = ctx.enter_context(tc.tile_pool(name="ps_sc", bufs=2, space="PSUM"))
        ps_yt = ctx.enter_context(tc.tile_pool(name="ps_yt", bufs=1, space="PSUM"))

        # ---- x tile 0 first: the first q matmul waits on it, and every
        # dma_start costs ~600ns of sequencer issue time ----
        xt0 = xt_pool.tile([128, 8, 512], FP16, tag="xt", name="xt8")
        nc.sync.dma_start(
            out=xt0[:], in_=xT[:].rearrange("(c p) f -> p c f", c=8)[:, :, 0:512]
        )
        # ---- constants (consolidated single-issue DMAs) ----
        wq_sb = const.tile([128, 8, 512], FP16)
        wk_sb = const.tile([128, 8, 512], FP16)
        wv_sb = const.tile([128, 8, 512], FP16)
        nc.sync.dma_start(out=wq_sb[:], in_=wq[:].rearrange("(c p) f -> p c f", c=8))
        nc.sync.dma_start(out=wk_sb[:], in_=wk[:].rearrange("(c p) f -> p c f", c=8))
        nc.sync.dma_start(out=wv_sb[:], in_=wv[:].rearrange("(c p) f -> p c f", c=8))
        wp_sb = const.tile([128, 4, 1024], FP16)
        nc.sync.dma_start(out=wp_sb[:], in_=wp[:].rearrange("(c p) f -> p c f", c=4))
        cs1_sb = const.tile([128, S], FP16)
        nc.sync.dma_start(out=cs1_sb[:], in_=cs1[:])
        cs2_sb = const.tile([128, S], FP16)
        nc.sync.dma_start(out=cs2_sb[:], in_=cs2[:])
        tri_sb = const.tile([128, 256], FP16)
        nc.sync.dma_start(out=tri_sb[:], in_=tri2[:])
        ones_sb = const.tile([128, 64], FP16)
        nc.sync.dma_start(out=ones_sb[:], in_=ones1[:])

        # ---- persistent slabs ----
        qT = slab.tile([128, NHP, S], FP16)   # rows: pair dims [evens32|odds32]x2
        kT = slab.tile([128, NHP, S], FP16)
        v_sb = slab.tile([128, HPC, NKT, 128], FP16)  # [keys, head, kt, 64v|64ones]
        vflat = v_sb[:].rearrange("p h k c -> p (h k) c")
        nc.vector.tensor_copy(
            vflat[:, :, 64:128],
            ones_sb[:, None, :].broadcast_to([128, HPC * NKT, 64]),
        )

        def rope(dst, ps, t):
            """dst[128,512] fp16 = qs*cs1_t + blockswap(qs)*cs2_t. The swap is
            single-input copies (dual-SBUF-input ops must be partition-aligned
            on TRN2); the second mul + final add run on the idle GpSimd."""
            qs = rtmp.tile([128, 512], FP16, tag="qs", name="qs")
            nc.vector.tensor_copy(qs[:], ps[:])
            sw = rtmp.tile([128, 512], FP16, tag="sw", name="sw")
            for blk in range(4):
                nc.vector.tensor_copy(sw[ts(blk, 32), :], qs[ts(blk ^ 1, 32), :])
            t1 = rtmp.tile([128, 512], FP16, tag="t1", name="t1")
            t2 = rtmp.tile([128, 512], FP16, tag="t2", name="t2")
            nc.vector.tensor_mul(t1[:], qs[:], cs1_sb[:, ts(t, 512)])
            nc.vector.tensor_mul(t2[:], sw[:], cs2_sb[:, ts(t, 512)])
            nc.vector.tensor_add(dst, t1[:], t2[:])

        def make_p1_units(t, pre_xt=None):
            """Phase-1 (qkv) for token tile t, split into ~850ns PE units.
            Returns (front, rest): front = dma + q/k for head-pair 0 + all
            vT subtiles (what attention(t) hp0 needs); rest = q/k for head
            pairs 1-3, pumped during attention(t)'s earlier head pairs.
            Units are (fn, has_pe_work) pairs."""
            front, rest = [], []
            st = {}
            if pre_xt is not None:
                st["xt"] = pre_xt
            else:
                def u_dma():
                    xt8 = xt_pool.tile([128, 8, 512], FP16, tag="xt", name="xt8")
                    st["xt"] = xt8
                    nc.sync.dma_start(
                        out=xt8[:],
                        in_=xT[:].rearrange("(c p) f -> p c f", c=8)[:, :, ts(t, 512)],
                    )

                front.append((u_dma, False))

            def qk_units(w_sb, dstT, key, hp, out):
                def u_a(hp=hp):
                    ps = ps_qkv.tile([128, 512], F32, tag="qkv", name="qk_ps")
                    st[f"{key}{hp}"] = ps
                    for c in range(4):
                        nc.tensor.matmul(
                            ps[:], w_sb[:, c, ts(hp, 128)], st["xt"][:, c, :],
                            start=(c == 0), stop=False,
                        )

                def u_b(hp=hp):
                    ps = st[f"{key}{hp}"]
                    for c in range(4, 8):
                        nc.tensor.matmul(
                            ps[:], w_sb[:, c, ts(hp, 128)], st["xt"][:, c, :],
                            start=False, stop=(c == 7),
                        )

                def u_r(hp=hp):
                    rope(dstT[:, hp, ts(t, 512)], st[f"{key}{hp}"], t)

                out.extend([(u_a, True), (u_b, True), (u_r, False)])

            qk_units(wq_sb, qT, "q", 0, front)
            qk_units(wk_sb, kT, "k", 0, front)

            # vT chunks: per token-sub s, out [128 tokens, 512 vdims]
            for s in range(4):
                def v_a(s=s):
                    ps = ps_qkv.tile([128, 512], F32, tag="qkv", name="vT_ps")
                    st[f"vps{s}"] = ps
                    for c in range(4):
                        nc.tensor.matmul(
                            ps[:], st["xt"][:, c, ts(s, 128)], wv_sb[:, c, :],
                            start=(c == 0), stop=False,
                        )

                def v_b(s=s):
                    ps = st[f"vps{s}"]
                    for c in range(4, 8):
                        nc.tensor.matmul(
                            ps[:], st["xt"][:, c, ts(s, 128)], wv_sb[:, c, :],
                            start=False, stop=(c == 7),
                        )

                def v_c(s=s):
                    kt = t * 4 + s
                    src = st[f"vps{s}"][:].rearrange("p (h d) -> p h d", h=HPC)
                    nc.vector.tensor_copy(v_sb[:, :, kt, 0:64], src)

                front.extend([(v_a, True), (v_b, True), (v_c, False)])

            for hp in range(1, NHP):
                qk_units(wq_sb, qT, "q", hp, rest)
                qk_units(wk_sb, kT, "k", hp, rest)
            return front, rest

        consumed = [0]

        def pump(feed, n_pe):
            """Run feed units until n_pe PE-bearing units have been emitted
            (light DVE-only units don't count toward the budget)."""
            while n_pe > 0 and feed:
                fn, has_pe = feed.popleft()
                fn()
                consumed[0] += 1
                if has_pe:
                    n_pe -= 1

        def attention(j, yTn, feed, gates=None):
            nkt = 4 * (j + 1)
            tri3 = tri_sb[:].rearrange("p (h w) -> p h w", h=2)
            for hp in range(NHP):
                if gates is not None and hp >= 1:
                    # hp's scores read qT/kT[hp] of tile j, produced by carry
                    # units still in the feed — force-run them before emitting
                    # any reader (tile deps are ordered by emission).
                    while consumed[0] < gates[hp] and feed:
                        fn, _ = feed.popleft()
                        fn()
                        consumed[0] += 1
                yts = ps_yt.tile([128, 1024], F32, tag="yt", name="yts")
                exs = {}

                def av(kt, j=j, hp=hp, yts=yts, exs=exs):
                    d = kt - 4 * j
                    off = 128 * d if d > 0 else 0
                    for h in range(2):
                        nc.tensor.matmul(
                            yts[:, h * 512 + off : (h + 1) * 512],
                            v_sb[:, 2 * hp + h, kt, :],
                            exs[kt][:, h * 512 + off : (h + 1) * 512],
                            start=(kt == 0), stop=(kt == nkt - 1),
                            skip_group_check=True,
                        )

                for kt in range(nkt):
                    d = kt - 4 * j
                    off = 128 * d if d > 0 else 0
                    sc = ps_sc.tile([128, 1024], F32, tag="sc", name="sc")
                    for h in range(2):
                        nc.tensor.matmul(
                            sc[:, h * 512 + off : (h + 1) * 512],
                            kT[ts(h, 64), hp, ts(kt, 128)],
                            qT[ts(h, 64), hp, j * 512 + off : (j + 1) * 512],
                            start=True, stop=True,
                        )
                    ex = ex_pool.tile([128, 1024], FP16, tag="ex", name="ex")
                    if off == 0:
                        nc.scalar.activation(ex[:], sc[:], AF.Exp, scale=0.125)
                    else:
                        sc3 = sc[:].rearrange("p (h w) -> p h w", h=2)
                        ex3 = ex[:].rearrange("p (h w) -> p h w", h=2)
                        nc.scalar.activation(
                            ex3[:, :, off:], sc3[:, :, off:], AF.Exp, scale=0.125
                        )
                    if d >= 0:
                        ex3 = ex[:].rearrange("p (h w) -> p h w", h=2)
                        nc.vector.tensor_mul(
                            ex3[:, :, off : off + 128],
                            ex3[:, :, off : off + 128],
                            tri3[:],
                        )
                    exs[kt] = ex
                    if debug and j == 1 and hp == 0:
                        nc.sync.dma_start(
                            out=dbg_ex[:, kt * 1024 : (kt + 1) * 1024], in_=ex[:]
                        )
                    if kt >= 2:
                        av(kt - 2)
                    pump(feed, 1)
                av(nkt - 2)
                av(nkt - 1)

                rc = rc_pool.tile([64, 1024], F32, tag="rc", name="rc")
                if debug and j == 1 and hp == 0:
                    ys = rtmp.tile([128, 1024], F32, tag="dbgys", name="dbgys")
                    nc.vector.tensor_copy(ys[:], yts[:])
                    nc.sync.dma_start(out=dbg_yts[:], in_=ys[:])
                # reciprocal_approx_fast's BITWISE_NOT seed needs exact fp32
                # bits; PSUM reads don't preserve them — stage through SBUF
                # (on ACT: it has slack, DVE is loaded).
                den = rc_pool.tile([64, 1024], F32, tag="den", name="den")
                nc.scalar.copy(den[:], yts[64:128, :])
                nc.vector.reciprocal_approx_fast(rc[:], den[:])
                if debug and j == 1 and hp == 0:
                    nc.sync.dma_start(out=dbg_rc[:], in_=rc[:])
                for h in range(2):
                    nc.vector.tensor_mul(
                        yTn[ts(h, 64), hp, :],
                        yts[0:64, ts(h, 512)],
                        rc[:, ts(h, 512)],
                    )

        def proj_units(j, yTn):
            units = []
            for dt in range(8):
                def u(dt=dt, yTn=yTn):
                    po = ps_qkv.tile([128, 512], F32, tag="qkv", name="po")
                    for hp in range(NHP):
                        nc.tensor.matmul(
                            po[:], wp_sb[:, hp, ts(dt, 128)], yTn[:, hp, :],
                            start=(hp == 0), stop=(hp == NHP - 1),
                        )
                    ob = ob_pool.tile([128, 512], FP16, tag="ob", name="ob")
                    nc.scalar.copy(ob[:], po[:])
                    nc.sync.dma_start(
                        out=outT[ts(dt, 128), ts(j, 512)], in_=ob[:]
                    )

                units.append((u, True))
            return units

        # ---- main schedule ----
        yTns = {}
        front0, rest0 = make_p1_units(0, pre_xt=xt0)
        for fn, _ in front0:
            fn()
        carry = list(rest0)
        for t in range(1, NT + 1):
            j = t - 1
            yTns[j] = ytn_pool.tile([128, NHP, 512], FP16, tag="yTn", name="yTn")
            feed = deque(carry)
            carry = []
            if t <= NT - 1:
                front, rest = make_p1_units(t)
                feed.extend(front)
                carry = list(rest)
            if j >= 1:
                feed.extend(proj_units(j - 1, yTns[j - 1]))
            consumed[0] = 0
            # carry units sit at the front of the feed: 6 per head pair
            attention(j, yTns[j], feed, gates=[0, 6, 12, 18])
            while feed:
                fn, _ = feed.popleft()
                fn()
        for fn, _ in proj_units(NT - 1, yTns[NT - 1]):
            fn()
        if debug:
            nc.sync.dma_start(
                out=dbg_q[:], in_=qT[:].rearrange("p h s -> p (h s)")
            )
            nc.sync.dma_start(
                out=dbg_k[:], in_=kT[:].rearrange("p h s -> p (h s)")
            )
            nc.sync.dma_start(
                out=dbg_v[:], in_=v_sb[:].rearrange("p h k c -> p (h k c)")
            )
            for j in range(4):
                nc.sync.dma_start(
                    out=dbg_y[:, j * NHP * 512 : (j + 1) * NHP * 512],
                    in_=yTns[j][:].rearrange("p h s -> p (h s)"),
                )
    nc.finalize()
    return nc


# ---------------- host side ----------------

def host_prepare(x, W_qkv, W_proj):
    x = np.asarray(x, dtype=np.float32)
    Wq = np.asarray(W_qkv[:, 0:D], dtype=np.float32)
    Wk = np.asarray(W_qkv[:, D : 2 * D], dtype=np.float32)
    Wv = np.asarray(W_qkv[:, 2 * D : 3 * D], dtype=np.float32)
    Wp = np.asarray(W_proj, dtype=np.float32)
    perm = np.concatenate([np.arange(0, DH, 2), np.arange(1, DH, 2)])
    half = DH // 2
    inv_freq = 1.0 / (10000.0 ** (np.arange(half, dtype=np.float64) / half))
    freqs = np.outer(np.arange(S, dtype=np.float64), inv_freq)
    cosT = np.cos(freqs).T
    sinT = np.sin(freqs).T
    # rope: dst = qs*cs1 + swap(qs)*cs2
    cs1 = np.concatenate([cosT, cosT, cosT, cosT], axis=0).astype(np.float16)
    cs2s = np.concatenate([-sinT, sinT, -sinT, sinT], axis=0).astype(np.float16)
    ii = np.arange(128)[:, None]
    qq = np.arange(128)[None, :]
    tri = (ii <= qq).astype(np.float16)
    tri2 = np.concatenate([tri, tri], axis=1)

    def headcols(W, h, p=None):
        w = W[:, h * DH : (h + 1) * DH]
        return w[:, p] if p is not None else w

    in_maps = []
    for c in range(NCORE):
        b, hg = c // 2, c % 2
        heads = [8 * hg + i for i in range(HPC)]
        wq_c = np.concatenate([headcols(Wq, h, perm) for h in heads], axis=1)
        wk_c = np.concatenate([headcols(Wk, h, perm) for h in heads], axis=1)
        wv_c = np.concatenate([headcols(Wv, h) for h in heads], axis=1)
        wp_c = Wp[heads[0] * DH : (heads[-1] + 1) * DH, :]
        in_maps.append(
            {
                "xT": np.ascontiguousarray(x[b].T).astype(np.float16),
                "wq": np.ascontiguousarray(wq_c).astype(np.float16),
                "wk": np.ascontiguousarray(wk_c).astype(np.float16),
                "wv": np.ascontiguousarray(wv_c).astype(np.float16),
                "wp": np.ascontiguousarray(wp_c).astype(np.float16),
                "cs1": cs1,
                "cs2": cs2s,
                "tri2": tri2,
                "ones1": np.ones((128, 64), dtype=np.float16),
            }
        )
    return in_maps


def _gather(res):
    out = np.empty((B, S, D), dtype=np.float32)
    for b in range(B):
        acc = res.results[2 * b]["outT"].astype(np.float32) + res.results[
            2 * b + 1
        ]["outT"].astype(np.float32)
        out[b] = acc.T
    return out


def kernel(x, W_qkv, W_proj):
    """Grading entrypoint: full inputs in, full output out.

    x [4, 2048, 1024] fp32, W_qkv [1024, 3072] fp32, W_proj [1024, 1024] fp32
    -> [4, 2048, 1024] fp32
    """
    from concourse.bass_utils import run_bass_kernel_spmd

    in_maps = host_prepare(np.asarray(x), np.asarray(W_qkv), np.asarray(W_proj))
    nc = build()
    res = run_bass_kernel_spmd(nc, in_maps, list(range(NCORE)))
    return _gather(res)


def kernel_traced(x, W_qkv, W_proj, trace=False):
    """Dev helper: also returns the BassKernelResults (exec_time_ns etc.)."""
    from concourse.bass_utils import run_bass_kernel_spmd

    in_maps = host_prepare(np.asarray(x), np.asarray(W_qkv), np.asarray(W_proj))
    nc = build()
    res = run_bass_kernel_spmd(nc, in_maps, list(range(NCORE)), trace=trace)
    return _gather(res), res
